# revision 7
# baseline (speedup 1.0000x reference)
"""Trainium2 Bass kernel for GaussianKernelGCNLayer (v8: fused chunk-major).

Reference computation (per instance b of 2048 = 8*256):
  wf[b,k,d] = sum_n w[b,n,k] * f[b,n,d]         (n=32 neighbors, k=8 kernels)
  out[b,k,o] = sum_d wf[b,k,d] * CW[k,d,o]      (d=4096, o=512)

Sharding: data-parallel over the 2048 instances -> 256 per core on 8 cores.

Per-core schedule (all matmul inputs bf16, fp32 PSUM accumulate):
  Pass 0 (chunk-major, fused): for each 128-wide d-chunk c, half-groups h:
    - DMA features chunk c, groups h*32..h*32+32 ([128 (bi n), 32 g, 128 d],
      host-prepacked so each partition reads one contiguous 8KB run),
      alternating between the SP and ACT HWDGE rings.
    - Phase 1: per group one matmul (lhsT = feature block [128, 128],
      rhs = block-diag weights [128, (k bi)=32]) -> psum [128 d, g, (k bi)],
      evacuated to wfT[:, c*8+k, b] with ScalarE/VectorE alternating.
    - Phase 2 for k=0,1: 4 accumulating matmuls (k x mtile) per chunk into
      4 bank-resident psum accumulators, hidden under the feature DMA.
  Passes 1-3: phase 2 for k-pairs (2,3), (4,5), (6,7): k-outer over chunks,
  lhsT = wfT slices (contiguous -> fast weight loads), rhs = CW tiles
  streamed chunk-major ([c, p, k, o] host layout).

PSUM: phase-1 tiles 2 banks x 2 bufs + 4 persistent accumulator banks = 8.
"""

import os
import sys

import numpy as np

try:
    import ml_dtypes
except ImportError:  # pragma: no cover
    ml_dtypes = None

for _p in ("/opt/trn_rl_repo",):
    if _p not in sys.path:
        sys.path.insert(0, _p)

NB, NI, NN, DIN = 8, 256, 32, 4096
NK, DKO = 8, 512
NCORES = 8
BL = NB * NI // NCORES  # 256 instances per core
NGRP = BL // 4          # 64 groups of 4 instances
NCH = DIN // 128        # 32 d-chunks
BF16 = ml_dtypes.bfloat16 if ml_dtypes is not None else None

_cached_nc = None


def _build(repeat=1, phases=(1, 2)):
    from contextlib import ExitStack

    import concourse.bass as bass  # noqa: F401
    import concourse.tile as tile
    from concourse import bacc, mybir

    nc = bacc.Bacc(
        "TRN2",
        target_bir_lowering=False,
        debug=False,
        num_devices=NCORES,
    )

    # features, chunk-major halves: [c, (bi n), g, dsub]
    f_d = nc.dram_tensor(
        "fstack", [NCH, 128, NGRP, 128], mybir.dt.bfloat16, kind="ExternalInput"
    ).ap()
    w_d = nc.dram_tensor(
        "wblk", [128, NGRP, NK, 4], mybir.dt.bfloat16, kind="ExternalInput"
    ).ap()
    # conv weights, chunk-major: [c, p, k, o]
    cw_d = nc.dram_tensor(
        "cw", [NCH, 128, NK, DKO], mybir.dt.bfloat16, kind="ExternalInput"
    ).ap()
    out_d = nc.dram_tensor(
        "out", [BL, NK * DKO], mybir.dt.bfloat16, kind="ExternalOutput"
    ).ap()

    with ExitStack() as ctx:
        tc = ctx.enter_context(tile.TileContext(nc))
        const_pool = ctx.enter_context(tc.tile_pool(name="const", bufs=1))
        fpool = ctx.enter_context(tc.tile_pool(name="fpool", bufs=5))
        ps1 = ctx.enter_context(tc.tile_pool(name="ps1", bufs=2, space="PSUM"))
        ps2 = ctx.enter_context(tc.tile_pool(name="ps2", bufs=1, space="PSUM"))
        wtpool = ctx.enter_context(tc.tile_pool(name="wtpool", bufs=3))
        opool = ctx.enter_context(tc.tile_pool(name="opool", bufs=2))

        # Persistent transposed wf: [128 (d%128), (c k), b] bf16.
        wfT = const_pool.tile(
            [128, NCH * NK, BL], mybir.dt.bfloat16, name="wfT"
        )
        wball = const_pool.tile(
            [128, NGRP, NK, 4], mybir.dt.bfloat16, name="wball"
        )

        if repeat > 1:
            ctx.enter_context(tc.For_i(0, repeat, 1))

        nc.sync.dma_start(wball[:], w_d)

        if 1 not in phases:
            # timing-only phase-2 build: fill wfT with garbage features
            nc.sync.dma_start(
                wfT[:].rearrange("p a b -> p (a b)"),
                f_d.rearrange("c p g d -> p (c g d)")[:, 0 : NCH * NK * BL],
            )

        def p2_chunk(k, c, wt_slice, pok):
            for mt in range(2):
                nc.tensor.matmul(
                    pok[mt][:],
                    wfT[:, c * NK + k, mt * 128 : (mt + 1) * 128],
                    wt_slice,
                    start=(c == 0),
                    stop=(c == NCH - 1),
                )

        def p2_finish(k, pok):
            for mt in range(2):
                ot = opool.tile([128, DKO], mybir.dt.bfloat16, name="ot")
                if mt == 0:
                    nc.vector.tensor_copy(ot[:], pok[mt][:])
                else:
                    nc.scalar.copy(ot[:], pok[mt][:])
                nc.scalar.dma_start(
                    out_d[mt * 128 : (mt + 1) * 128, k * DKO : (k + 1) * DKO],
                    ot[:],
                )

        def acc_pair(pair, tag_extra=""):
            return {
                k: (
                    ps2.tile(
                        [128, DKO], mybir.dt.float32,
                        name=f"acc{2 * ki}", tag=f"acc{2 * ki}",
                    ),
                    ps2.tile(
                        [128, DKO], mybir.dt.float32,
                        name=f"acc{2 * ki + 1}", tag=f"acc{2 * ki + 1}",
                    ),
                )
                for ki, k in enumerate(pair)
            }

        # ---- Pass 0: chunk-major phase 1 fused with phase 2 of k=0,1 ----
        fused = (0, 1) if 2 in phases else ()
        if 1 in phases:
            po = acc_pair(fused)
            wts = {}
            for c in range(NCH):
                if fused:
                    wt = wtpool.tile(
                        [128, 2, DKO], mybir.dt.bfloat16, name="wt", tag="wt",
                    )
                    wts[c % 3] = wt
                    nc.sync.dma_start(
                        wt[:], cw_d[c, :, fused[0] : fused[-1] + 1, :]
                    )
                for h in range(2):
                    fs = fpool.tile([128, 32, 128], mybir.dt.bfloat16, name="fs")
                    src = f_d[c, :, h * 32 : (h + 1) * 32, :]
                    if (2 * c + h) % 2 == 0:
                        nc.sync.dma_start(fs[:], src)
                    else:
                        nc.scalar.dma_start(fs[:], src)
                    pt = ps1.tile([128, 32, 32], mybir.dt.float32, name="pt")
                    for gg in range(32):
                        g = h * 32 + gg
                        nc.tensor.matmul(
                            pt[:, gg, :],
                            fs[:, gg, :],
                            wball[:, g, :, :],
                            start=True,
                            stop=True,
                        )
                    # psum [128, g, (k bi)] -> wfT[:, c*8+k, (g bi)]
                    dst = wfT[
                        :, c * NK : (c + 1) * NK, h * 128 : (h + 1) * 128
                    ].rearrange("p k (g bi) -> p k g bi", bi=4)
                    srcp = pt[:].rearrange("p g (k bi) -> p k g bi", bi=4)
                    if (c + h) % 2 == 0:
                        nc.scalar.copy(dst, srcp)
                    else:
                        nc.vector.tensor_copy(dst, srcp)
                # consume chunk c-1 (lagged so PE never stalls on copies)
                if c > 0:
                    for ki, k in enumerate(fused):
                        p2_chunk(k, c - 1, wts[(c - 1) % 3][:, ki, :], po[k])
            for ki, k in enumerate(fused):
                p2_chunk(k, NCH - 1, wts[(NCH - 1) % 3][:, ki, :], po[k])
            for k in fused:
                p2_finish(k, po[k])

        # ---- Passes 1-3: phase 2 for remaining k-pairs ----
        rest = [k for k in range(NK if 2 in phases else 0) if k not in fused]
        for kp in range(0, len(rest), 2):
            pair = rest[kp : kp + 2]
            po2 = acc_pair(pair)
            for c in range(NCH):
                wt = wtpool.tile(
                    [128, 2, DKO], mybir.dt.bfloat16, name="wt", tag="wt",
                )
                if c % 2 == 0:
                    nc.sync.dma_start(
                        wt[:], cw_d[c, :, pair[0] : pair[-1] + 1, :]
                    )
                else:
                    nc.scalar.dma_start(
                        wt[:], cw_d[c, :, pair[0] : pair[-1] + 1, :]
                    )
                for ki, k in enumerate(pair):
                    p2_chunk(k, c, wt[:, ki, :], po2[k])
            for k in pair:
                p2_finish(k, po2[k])

    nc.compile()
    return nc


def _prep_inputs(neighbourhood_features, neighbourhood_weights, conv_weight):
    f = np.asarray(neighbourhood_features, dtype=np.float32).reshape(
        NB * NI, NN, DIN
    )
    w = np.asarray(neighbourhood_weights, dtype=np.float32).reshape(NB * NI, NN, NK)
    # cw chunk-major: [c, p, k, o]
    cw = np.asarray(conv_weight, dtype=np.float32).reshape(NK, NCH, 128, DKO)
    cw16 = np.ascontiguousarray(cw.transpose(1, 2, 0, 3)).astype(BF16)
    in_maps = []
    for i in range(NCORES):
        # features chunk-major: [c, (bi n), g, dsub]
        fl = (
            f[i * BL : (i + 1) * BL]
            .reshape(NGRP, 4 * NN, NCH, 128)
            .transpose(2, 1, 0, 3)
        )
        fl = np.ascontiguousarray(fl).astype(BF16)
        wl = w[i * BL : (i + 1) * BL].reshape(NGRP, 4, NN, NK)
        wblk = np.zeros((128, NGRP, NK, 4), dtype=np.float32)
        for bi in range(4):
            wblk[bi * NN : (bi + 1) * NN, :, :, bi] = wl[:, bi].transpose(1, 0, 2)
        in_maps.append(
            {
                "fstack": fl,
                "wblk": wblk.astype(BF16),
                "cw": cw16,
            }
        )
    return in_maps


def _execute(neighbourhood_features, neighbourhood_weights, conv_weight, trace=False):
    global _cached_nc
    if _cached_nc is None:
        _cached_nc = _build()
    nc = _cached_nc
    from concourse import bass_utils

    in_maps = _prep_inputs(
        neighbourhood_features, neighbourhood_weights, conv_weight
    )
    res = bass_utils.run_bass_kernel_spmd(
        nc, in_maps, core_ids=list(range(NCORES)), trace=trace
    )
    outs = [np.asarray(res.results[i]["out"], dtype=np.float32) for i in range(NCORES)]
    full = np.concatenate(outs, axis=0)
    return full.reshape(NB, NI, NK * DKO), res


def kernel(neighbourhood_features, neighbourhood_weights, conv_weight):
    out, _ = _execute(
        neighbourhood_features, neighbourhood_weights, conv_weight, trace=False
    )
    return out


# revision 9
# speedup vs baseline: 1.2385x; 1.2385x over previous
"""Trainium2 Bass kernel for GaussianKernelGCNLayer (v9: triple-ring fs DMA).

Reference computation (per instance b of 2048 = 8*256):
  wf[b,k,d] = sum_n w[b,n,k] * f[b,n,d]         (n=32 neighbors, k=8 kernels)
  out[b,k,o] = sum_d wf[b,k,d] * CW[k,d,o]      (d=4096, o=512)

Sharding: data-parallel over the 2048 instances -> 256 per core on 8 cores.

Per-core device algorithm (all matmul inputs bf16, fp32 PSUM accumulate):
  Phase 1: for each group g of 4 instances, the 4x32 neighbor features are
    stacked into a [128, 4096] SBUF tile (contract dim = 4*32 = 128
    partitions) and matmul'd against a host-prebuilt block-diagonal
    weight tile [128, (k bi)=32] -> psum [128(d), (k bi)]: wf TRANSPOSED
    (d on partitions), which is the layout phase 2 needs.  The psum->SBUF
    evacuation alternates between ScalarE and VectorE and is contiguous
    on both sides (k-major column order matches wfT's (c k) layout).
  Phase 2: for each kernel k: out[b, k*512:+512] = wf_k @ CW_k as 32
    accumulating matmuls over d-chunks; lhsT = wfT[:, c*8+k, mtile*128:+128]
    (contiguous -> FWL weight loads), rhs = CW[k, chunk] ([128 d, 512 o]).
    CW is host-transposed to [k, p, c, o] so each [128, 8, 512] tile is one
    descriptor per partition (8KB contiguous runs).
"""

import os
import sys

import numpy as np

try:
    import ml_dtypes
except ImportError:  # pragma: no cover
    ml_dtypes = None

for _p in ("/opt/trn_rl_repo",):
    if _p not in sys.path:
        sys.path.insert(0, _p)

NB, NI, NN, DIN = 8, 256, 32, 4096
NK, DKO = 8, 512
NCORES = 8
BL = NB * NI // NCORES  # 256 instances per core
NGRP = BL // 4          # 64 groups of 4 instances
NCH = DIN // 128        # 32 d-chunks
CWQ = 8                 # cw DMA granularity: chunks per DMA (1MB transfers)
BF16 = ml_dtypes.bfloat16 if ml_dtypes is not None else None

_cached_nc = None


def _build(repeat=1, phases=(1, 2)):
    from contextlib import ExitStack

    import concourse.bass as bass  # noqa: F401
    import concourse.tile as tile
    from concourse import bacc, mybir

    nc = bacc.Bacc(
        "TRN2",
        target_bir_lowering=False,
        debug=False,
        num_devices=NCORES,
    )

    f_d = nc.dram_tensor(
        "fstack", [NGRP, 128, DIN], mybir.dt.bfloat16, kind="ExternalInput"
    ).ap()
    w_d = nc.dram_tensor(
        "wblk", [128, NGRP, NK, 4], mybir.dt.bfloat16, kind="ExternalInput"
    ).ap()
    cw_d = nc.dram_tensor(
        "cw", [NK, 128, NCH, DKO], mybir.dt.bfloat16, kind="ExternalInput"
    ).ap()
    out_d = nc.dram_tensor(
        "out", [BL, NK * DKO], mybir.dt.bfloat16, kind="ExternalOutput"
    ).ap()

    NCW = NCH // CWQ  # cw DMAs per kernel k

    with ExitStack() as ctx:
        tc = ctx.enter_context(tile.TileContext(nc))
        const_pool = ctx.enter_context(tc.tile_pool(name="const", bufs=1))
        fpool = ctx.enter_context(tc.tile_pool(name="fpool", bufs=4))
        ps1 = ctx.enter_context(tc.tile_pool(name="ps1", bufs=3, space="PSUM"))
        ps2 = ctx.enter_context(tc.tile_pool(name="ps2", bufs=4, space="PSUM"))
        wtpool = ctx.enter_context(tc.tile_pool(name="wtpool", bufs=3))
        opool = ctx.enter_context(tc.tile_pool(name="opool", bufs=4))

        # Persistent transposed wf: [128 (d%128), (c k), b] bf16.
        # Phase-2 lhsT = wfT[:, c*8+k, mt*128:(mt+1)*128] is contiguous.
        wfT = const_pool.tile(
            [128, NCH * NK, BL], mybir.dt.bfloat16, name="wfT"
        )
        # Block-diag neighbour weights, all groups resident: [128, g, k, bi]
        wball = const_pool.tile(
            [128, NGRP, NK, 4], mybir.dt.bfloat16, name="wball"
        )

        if repeat > 1:
            ctx.enter_context(tc.For_i(0, repeat, 1))

        nc.sync.dma_start(wball[:], w_d)

        if 1 not in phases:
            # timing-only phase-2 build: fill wfT with garbage features
            nc.sync.dma_start(
                wfT[:].rearrange("p a b -> p (a b)").rearrange("p (g d) -> p g d", g=16), f_d[0:16, :, :].rearrange("g p d -> p g d")
            )

        # ---- Phase 1: wfT[d, (c k), b] per instance-group ----
        for g in range(NGRP if 1 in phases else 0):
            fs = fpool.tile([128, DIN], mybir.dt.bfloat16, name="fs")
            if g % 3 == 0:
                nc.sync.dma_start(fs[:], f_d[g, :, :])
            elif g % 3 == 1:
                nc.scalar.dma_start(fs[:], f_d[g, :, :])
            else:
                nc.gpsimd.dma_start(fs[:], f_d[g, :, :])
            for h in range(2):
                pt = ps1.tile([128, 128, 4], mybir.dt.float32, name="pt")
                for cc in range(16):
                    c = h * 16 + cc
                    nc.tensor.matmul(
                        pt[:, cc * 8 : (cc + 1) * 8, :],
                        fs[:, c * 128 : (c + 1) * 128],
                        wball[:, g, :, :],
                        start=True,
                        stop=True,
                    )
                # psum [128, (cc k), bi] -> wfT[:, h*128:(h+1)*128, g*4:+4]
                dst = wfT[:, h * 128 : (h + 1) * 128, g * 4 : (g + 1) * 4]
                if (g + h) % 2 == 0:
                    nc.scalar.copy(dst, pt[:])
                else:
                    nc.vector.tensor_copy(dst, pt[:])

        # ---- Phase 2: out = wf @ CW, k-outer, both m-tiles per W pass ----
        for k in range(NK if 2 in phases else 0):
            po0 = ps2.tile([128, DKO], mybir.dt.float32, name="po0", tag="po")
            po1 = ps2.tile([128, DKO], mybir.dt.float32, name="po1", tag="po")
            pos = (po0, po1)
            for q in range(NCW):
                wt = wtpool.tile([128, CWQ, DKO], mybir.dt.bfloat16, name="wt")
                nc.sync.dma_start(wt[:], cw_d[k, :, q * CWQ : (q + 1) * CWQ, :])
                for cc in range(CWQ):
                    c = q * CWQ + cc
                    for mt in range(2):
                        nc.tensor.matmul(
                            pos[mt][:],
                            wfT[:, c * NK + k, mt * 128 : (mt + 1) * 128],
                            wt[:, cc, :],
                            start=(c == 0),
                            stop=(c == NCH - 1),
                        )
            for mt in range(2):
                ot = opool.tile([128, DKO], mybir.dt.bfloat16, name="ot")
                if mt == 0:
                    nc.vector.tensor_copy(ot[:], pos[mt][:])
                else:
                    nc.scalar.copy(ot[:], pos[mt][:])
                nc.scalar.dma_start(
                    out_d[mt * 128 : (mt + 1) * 128, k * DKO : (k + 1) * DKO],
                    ot[:],
                )

    nc.compile()
    return nc


def _prep_inputs(neighbourhood_features, neighbourhood_weights, conv_weight):
    f = np.asarray(neighbourhood_features, dtype=np.float32).reshape(
        NB * NI, NN, DIN
    )
    w = np.asarray(neighbourhood_weights, dtype=np.float32).reshape(NB * NI, NN, NK)
    # cw host-transposed: [k, p, c, o] with p = row % 128, c = row // 128
    cw = np.asarray(conv_weight, dtype=np.float32).reshape(NK, NCH, 128, DKO)
    cw16 = np.ascontiguousarray(cw.transpose(0, 2, 1, 3)).astype(BF16)
    in_maps = []
    for i in range(NCORES):
        fl = (
            f[i * BL : (i + 1) * BL]
            .reshape(NGRP, 4 * NN, DIN)
            .astype(BF16)
        )
        wl = w[i * BL : (i + 1) * BL].reshape(NGRP, 4, NN, NK)
        # wblk[(bi n), g, k, bi'] = w[g, bi, n, k] iff bi == bi'
        wblk = np.zeros((128, NGRP, NK, 4), dtype=np.float32)
        for bi in range(4):
            # wl[:, bi]: [g, n, k] -> [n, g, k]
            wblk[bi * NN : (bi + 1) * NN, :, :, bi] = wl[:, bi].transpose(1, 0, 2)
        in_maps.append(
            {
                "fstack": np.ascontiguousarray(fl),
                "wblk": wblk.astype(BF16),
                "cw": cw16,
            }
        )
    return in_maps


def _execute(neighbourhood_features, neighbourhood_weights, conv_weight, trace=False):
    global _cached_nc
    if _cached_nc is None:
        _cached_nc = _build()
    nc = _cached_nc
    from concourse import bass_utils

    in_maps = _prep_inputs(
        neighbourhood_features, neighbourhood_weights, conv_weight
    )
    res = bass_utils.run_bass_kernel_spmd(
        nc, in_maps, core_ids=list(range(NCORES)), trace=trace
    )
    outs = [np.asarray(res.results[i]["out"], dtype=np.float32) for i in range(NCORES)]
    full = np.concatenate(outs, axis=0)
    return full.reshape(NB, NI, NK * DKO), res


def kernel(neighbourhood_features, neighbourhood_weights, conv_weight):
    out, _ = _execute(
        neighbourhood_features, neighbourhood_weights, conv_weight, trace=False
    )
    return out


# revision 10
# speedup vs baseline: 1.3445x; 1.0856x over previous
"""Trainium2 Bass kernel for GaussianKernelGCNLayer (v6: deeper fs double-buffering).

Reference computation (per instance b of 2048 = 8*256):
  wf[b,k,d] = sum_n w[b,n,k] * f[b,n,d]         (n=32 neighbors, k=8 kernels)
  out[b,k,o] = sum_d wf[b,k,d] * CW[k,d,o]      (d=4096, o=512)

Sharding: data-parallel over the 2048 instances -> 256 per core on 8 cores.

Per-core device algorithm (all matmul inputs bf16, fp32 PSUM accumulate):
  Phase 1: for each group g of 4 instances, the 4x32 neighbor features are
    stacked into a [128, 4096] SBUF tile (contract dim = 4*32 = 128
    partitions) and matmul'd against a host-prebuilt block-diagonal
    weight tile [128, (k bi)=32] -> psum [128(d), (k bi)]: wf TRANSPOSED
    (d on partitions), which is the layout phase 2 needs.  The psum->SBUF
    evacuation alternates between ScalarE and VectorE and is contiguous
    on both sides (k-major column order matches wfT's (c k) layout).
  Phase 2: for each kernel k: out[b, k*512:+512] = wf_k @ CW_k as 32
    accumulating matmuls over d-chunks; lhsT = wfT[:, c*8+k, mtile*128:+128]
    (contiguous -> FWL weight loads), rhs = CW[k, chunk] ([128 d, 512 o]).
    CW is host-transposed to [k, p, c, o] so each [128, 8, 512] tile is one
    descriptor per partition (8KB contiguous runs).
"""

import os
import sys

import numpy as np

try:
    import ml_dtypes
except ImportError:  # pragma: no cover
    ml_dtypes = None

for _p in ("/opt/trn_rl_repo",):
    if _p not in sys.path:
        sys.path.insert(0, _p)

NB, NI, NN, DIN = 8, 256, 32, 4096
NK, DKO = 8, 512
NCORES = 8
BL = NB * NI // NCORES  # 256 instances per core
NGRP = BL // 4          # 64 groups of 4 instances
NCH = DIN // 128        # 32 d-chunks
CWQ = 8                 # cw DMA granularity: chunks per DMA (1MB transfers)
BF16 = ml_dtypes.bfloat16 if ml_dtypes is not None else None

_cached_nc = None


def _build(repeat=1, phases=(1, 2)):
    from contextlib import ExitStack

    import concourse.bass as bass  # noqa: F401
    import concourse.tile as tile
    from concourse import bacc, mybir

    nc = bacc.Bacc(
        "TRN2",
        target_bir_lowering=False,
        debug=False,
        num_devices=NCORES,
    )

    f_d = nc.dram_tensor(
        "fstack", [NGRP, 128, DIN], mybir.dt.bfloat16, kind="ExternalInput"
    ).ap()
    w_d = nc.dram_tensor(
        "wblk", [128, NGRP, NK, 4], mybir.dt.bfloat16, kind="ExternalInput"
    ).ap()
    cw_d = nc.dram_tensor(
        "cw", [NK, 128, NCH, DKO], mybir.dt.bfloat16, kind="ExternalInput"
    ).ap()
    out_d = nc.dram_tensor(
        "out", [BL, NK * DKO], mybir.dt.bfloat16, kind="ExternalOutput"
    ).ap()

    NCW = NCH // CWQ  # cw DMAs per kernel k

    with ExitStack() as ctx:
        tc = ctx.enter_context(tile.TileContext(nc))
        const_pool = ctx.enter_context(tc.tile_pool(name="const", bufs=1))
        fpool = ctx.enter_context(tc.tile_pool(name="fpool", bufs=4))
        ps1 = ctx.enter_context(tc.tile_pool(name="ps1", bufs=3, space="PSUM"))
        ps2 = ctx.enter_context(tc.tile_pool(name="ps2", bufs=4, space="PSUM"))
        wtpool = ctx.enter_context(tc.tile_pool(name="wtpool", bufs=3))
        opool = ctx.enter_context(tc.tile_pool(name="opool", bufs=4))

        # Persistent transposed wf: [128 (d%128), (c k), b] bf16.
        # Phase-2 lhsT = wfT[:, c*8+k, mt*128:(mt+1)*128] is contiguous.
        wfT = const_pool.tile(
            [128, NCH * NK, BL], mybir.dt.bfloat16, name="wfT"
        )
        # Block-diag neighbour weights, all groups resident: [128, g, k, bi]
        wball = const_pool.tile(
            [128, NGRP, NK, 4], mybir.dt.bfloat16, name="wball"
        )

        if repeat > 1:
            ctx.enter_context(tc.For_i(0, repeat, 1))

        nc.sync.dma_start(wball[:], w_d)

        if 1 not in phases:
            # timing-only phase-2 build: fill wfT with garbage features
            nc.sync.dma_start(
                wfT[:].rearrange("p a b -> p (a b)").rearrange("p (g d) -> p g d", g=16), f_d[0:16, :, :].rearrange("g p d -> p g d")
            )

        # ---- Phase 1: wfT[d, (c k), b] per instance-group ----
        for g in range(NGRP if 1 in phases else 0):
            fs = fpool.tile([128, DIN], mybir.dt.bfloat16, name="fs")
            if g % 2 == 0:
                nc.sync.dma_start(fs[:], f_d[g, :, :])
            else:
                nc.scalar.dma_start(fs[:], f_d[g, :, :])
            for h in range(2):
                pt = ps1.tile([128, 128, 4], mybir.dt.float32, name="pt")
                for cc in range(16):
                    c = h * 16 + cc
                    nc.tensor.matmul(
                        pt[:, cc * 8 : (cc + 1) * 8, :],
                        fs[:, c * 128 : (c + 1) * 128],
                        wball[:, g, :, :],
                        start=True,
                        stop=True,
                    )
                # psum [128, (cc k), bi] -> wfT[:, h*128:(h+1)*128, g*4:+4]
                dst = wfT[:, h * 128 : (h + 1) * 128, g * 4 : (g + 1) * 4]
                if (g + h) % 2 == 0:
                    nc.scalar.copy(dst, pt[:])
                else:
                    nc.vector.tensor_copy(dst, pt[:])

        # ---- Phase 2: out = wf @ CW, k-outer, both m-tiles per W pass ----
        for k in range(NK if 2 in phases else 0):
            po0 = ps2.tile([128, DKO], mybir.dt.float32, name="po0", tag="po")
            po1 = ps2.tile([128, DKO], mybir.dt.float32, name="po1", tag="po")
            pos = (po0, po1)
            for q in range(NCW):
                wt = wtpool.tile([128, CWQ, DKO], mybir.dt.bfloat16, name="wt")
                nc.sync.dma_start(wt[:], cw_d[k, :, q * CWQ : (q + 1) * CWQ, :])
                for cc in range(CWQ):
                    c = q * CWQ + cc
                    for mt in range(2):
                        nc.tensor.matmul(
                            pos[mt][:],
                            wfT[:, c * NK + k, mt * 128 : (mt + 1) * 128],
                            wt[:, cc, :],
                            start=(c == 0),
                            stop=(c == NCH - 1),
                        )
            for mt in range(2):
                ot = opool.tile([128, DKO], mybir.dt.bfloat16, name="ot")
                if mt == 0:
                    nc.vector.tensor_copy(ot[:], pos[mt][:])
                else:
                    nc.scalar.copy(ot[:], pos[mt][:])
                nc.scalar.dma_start(
                    out_d[mt * 128 : (mt + 1) * 128, k * DKO : (k + 1) * DKO],
                    ot[:],
                )

    nc.compile()
    return nc


def _prep_inputs(neighbourhood_features, neighbourhood_weights, conv_weight):
    f = np.asarray(neighbourhood_features, dtype=np.float32).reshape(
        NB * NI, NN, DIN
    )
    w = np.asarray(neighbourhood_weights, dtype=np.float32).reshape(NB * NI, NN, NK)
    # cw host-transposed: [k, p, c, o] with p = row % 128, c = row // 128
    cw = np.asarray(conv_weight, dtype=np.float32).reshape(NK, NCH, 128, DKO)
    cw16 = np.ascontiguousarray(cw.transpose(0, 2, 1, 3)).astype(BF16)
    in_maps = []
    for i in range(NCORES):
        fl = (
            f[i * BL : (i + 1) * BL]
            .reshape(NGRP, 4 * NN, DIN)
            .astype(BF16)
        )
        wl = w[i * BL : (i + 1) * BL].reshape(NGRP, 4, NN, NK)
        # wblk[(bi n), g, k, bi'] = w[g, bi, n, k] iff bi == bi'
        wblk = np.zeros((128, NGRP, NK, 4), dtype=np.float32)
        for bi in range(4):
            # wl[:, bi]: [g, n, k] -> [n, g, k]
            wblk[bi * NN : (bi + 1) * NN, :, :, bi] = wl[:, bi].transpose(1, 0, 2)
        in_maps.append(
            {
                "fstack": np.ascontiguousarray(fl),
                "wblk": wblk.astype(BF16),
                "cw": cw16,
            }
        )
    return in_maps


def _execute(neighbourhood_features, neighbourhood_weights, conv_weight, trace=False):
    global _cached_nc
    if _cached_nc is None:
        _cached_nc = _build()
    nc = _cached_nc
    from concourse import bass_utils

    in_maps = _prep_inputs(
        neighbourhood_features, neighbourhood_weights, conv_weight
    )
    res = bass_utils.run_bass_kernel_spmd(
        nc, in_maps, core_ids=list(range(NCORES)), trace=trace
    )
    outs = [np.asarray(res.results[i]["out"], dtype=np.float32) for i in range(NCORES)]
    full = np.concatenate(outs, axis=0)
    return full.reshape(NB, NI, NK * DKO), res


def kernel(neighbourhood_features, neighbourhood_weights, conv_weight):
    out, _ = _execute(
        neighbourhood_features, neighbourhood_weights, conv_weight, trace=False
    )
    return out
